# revision 21
# baseline (speedup 1.0000x reference)
"""Trainium2 Bass kernel for AssociativeMemoryModule (causal linear attention).

Sharding: head-parallel — core c owns head c for both batches. Each core:
  1. projects full x (pre-transposed on host) to [q.T;k.T] and v.T for its head,
  2. applies phi = elu+1 via exp(min(.,0)) + relu(.),
  3. PE-transposes k_phi / v tiles to normal layout,
  4. runs chunked causal linear attention (C=128) in transposed-output form,
  5. writes per-head out.T blocks to a DRAM bounce in t-blocked layout,
  6. one AllToAll redistributes head-sharded -> t-sharded,
  7. each core computes the o-projection for its 256-row t-slice.
Host gathers the 8 (512, 256) o.T slices and concatenates.
"""
import sys

import numpy as np

sys.path.insert(0, "/opt/trn_rl_repo")

H, HD, D = 8, 64, 512
B, T = 2, 1024
BT = B * T            # 2048
C = 128               # attention chunk
NCH = BT // C         # 16 chunks total
CPB = T // C          # 8 chunks per batch
TS = BT // 8          # 256: per-core output t-slice
NF = D // 128         # 4 feature tiles
NT = 4                # t-tiles of 512 for projections

_CACHE = {}


def _build():
    if "nc" in _CACHE:
        return _CACHE["nc"]
    import concourse.bass as bass
    import concourse.mybir as mybir
    import concourse.tile as tile
    from concourse import bacc
    from concourse.bass import ts

    f32 = mybir.dt.float32
    AF = mybir.ActivationFunctionType

    nc = bacc.Bacc("TRN2", target_bir_lowering=False, debug=False, num_devices=8)

    xT = nc.declare_dram_parameter("xT", [D, BT], f32, isOutput=False)
    wa = nc.declare_dram_parameter("wa", [D, 128], f32, isOutput=False)
    wv = nc.declare_dram_parameter("wv", [D, HD], f32, isOutput=False)
    wo = nc.declare_dram_parameter("wo", [D, D], f32, isOutput=False)
    bqk = nc.declare_dram_parameter("bqk", [128, 1], f32, isOutput=False)
    bv = nc.declare_dram_parameter("bv", [HD, 1], f32, isOutput=False)
    bo = nc.declare_dram_parameter("bo", [D, 1], f32, isOutput=False)
    out = nc.declare_dram_parameter("out", [D, TS], f32, isOutput=True)

    mask_np = np.triu(np.ones((C, C), np.float32))  # mask[s,t] = s<=t
    iden_np = np.eye(HD, dtype=np.float32)
    iden128_np = np.eye(C, dtype=np.float32)
    mask_d = nc.inline_tensor(mask_np, "causal_mask")
    iden_d = nc.inline_tensor(iden_np, "iden64")
    iden128_d = nc.inline_tensor(iden128_np, "iden128")

    with tile.TileContext(nc) as tc:
        with (
            tc.tile_pool(name="consts", bufs=1) as consts,
            tc.tile_pool(name="dram", bufs=1, space="DRAM") as dram,
        ):
            # ---- resident SBUF tensors ----
            xt_sb = consts.tile([128, NF, BT], f32)
            wa_sb = consts.tile([128, NF, 128], f32)
            wv_sb = consts.tile([128, NF, HD], f32)
            wo_sb = consts.tile([128, NF, D], f32)
            bqk_sb = consts.tile([128, 1], f32)
            bv_sb = consts.tile([HD, 1], f32)
            bo_sb = consts.tile([128, NF], f32)
            mask_sb = consts.tile([C, C], f32)
            iden_sb = consts.tile([HD, HD], f32)
            iden128_sb = consts.tile([C, C], f32)
            qk_phi = consts.tile([128, BT], f32)       # rows 0-63 qT, 64-127 kT
            k_sep = consts.tile([HD, BT], f32)         # kT re-based to partition 0
            vT_sb = consts.tile([HD, BT], f32)
            k_nrm = consts.tile([128, NCH, HD], f32)
            v_aug = consts.tile([128, NCH, HD + 1], f32)
            g_sb = consts.tile([128, NF, TS], f32)
            sm_all = consts.tile([C, NCH, C], f32)

            cc_in = dram.tile([8, HD, TS], f32)        # t-blocked out.T
            cc_out = dram.tile([8, HD, TS], f32)       # after A2A: head-blocked

            for f in range(NF):
                nc.sync.dma_start(wa_sb[:, f, :], wa[128 * f:128 * (f + 1), :])
                nc.sync.dma_start(wv_sb[:, f, :], wv[128 * f:128 * (f + 1), :])
            # xT split by (t-col, f) chunk so t-tile 0 becomes computable fast
            for tcol in range(NT):
                for f in range(NF):
                    nc.sync.dma_start(
                        xt_sb[:, f, ts(tcol, 512)],
                        xT[128 * f:128 * (f + 1), ts(tcol, 512)])
            for f in range(NF):
                nc.sync.dma_start(wo_sb[:, f, :], wo[128 * f:128 * (f + 1), :])
            nc.sync.dma_start(bqk_sb[:], bqk[:, :])
            nc.sync.dma_start(bv_sb[:], bv[:, :])
            nc.sync.dma_start(bo_sb[:], bo.ap().rearrange("(e p) o -> p (e o)", p=128))
            nc.sync.dma_start(mask_sb[:], mask_d[:, :])
            nc.sync.dma_start(iden_sb[:], iden_d[:, :])
            nc.sync.dma_start(iden128_sb[:], iden128_d[:, :])
            nc.vector.memset(v_aug[:, :, HD:HD + 1], 1.0)

            # ---- phase 1: projections + phi + transposes + scores ----
            with (
                tc.tile_pool(name="psA", bufs=2, space="PSUM") as psA,
                tc.tile_pool(name="psB", bufs=2, space="PSUM") as psB,
                tc.tile_pool(name="psT", bufs=2, space="PSUM") as psT,
                tc.tile_pool(name="psSc", bufs=2, space="PSUM") as psSc,
                tc.tile_pool(name="ptmp", bufs=2) as ptmp,
            ):
                for tt in range(NT):
                    sl = ts(tt, 512)
                    pa = psA.tile([128, 512], f32)
                    pb = psB.tile([HD, 512], f32)
                    for f in range(NF):
                        nc.tensor.matmul(pa, wa_sb[:, f, :], xt_sb[:, f, sl],
                                         start=(f == 0), stop=(f == NF - 1))
                    for f in range(NF):
                        nc.tensor.matmul(pb, wv_sb[:, f, :], xt_sb[:, f, sl],
                                         start=(f == 0), stop=(f == NF - 1))
                    qkr = ptmp.tile([128, 512], f32, tag="qkr")
                    nc.scalar.activation(qkr, pa, AF.Identity, bias=bqk_sb[:])
                    nc.scalar.activation(vT_sb[:, sl], pb, AF.Identity, bias=bv_sb[:])
                    mm = ptmp.tile([128, 512], f32, tag="mm")
                    rr = ptmp.tile([128, 512], f32, tag="rr")
                    ee = ptmp.tile([128, 512], f32, tag="ee")
                    nc.vector.tensor_scalar_min(mm, qkr, 0.0)
                    nc.scalar.activation(ee, mm, AF.Exp)
                    nc.vector.tensor_scalar_max(rr, qkr, 0.0)
                    nc.vector.tensor_add(qk_phi[:, sl], ee, rr)
                    # re-base kT rows 64-127 -> partition 0 (SBUF->SBUF DMA)
                    nc.sync.dma_start(k_sep[:, sl], qk_phi[64:128, sl])

                    # transposes + masked scores for the 4 chunks in this t-tile
                    for jj in range(4):
                        i = tt * 4 + jj
                        cs = ts(i, C)
                        pt = psT.tile([C, HD], f32, tag="tr")
                        nc.tensor.transpose(pt, k_sep[:, cs], iden_sb[:])
                        nc.scalar.copy(k_nrm[:, i, :], pt)
                        pv = psT.tile([C, HD], f32, tag="tr")
                        nc.tensor.transpose(pv, vT_sb[:, cs], iden_sb[:])
                        nc.vector.tensor_copy(v_aug[:, i, 0:HD], pv)
                        ps = psSc.tile([C, C], f32, tag="ps")
                        nc.tensor.matmul(ps, k_sep[:, cs], qk_phi[0:64, cs],
                                         start=True, stop=True)
                        nc.vector.tensor_mul(sm_all[:, i, :], ps, mask_sb[:])

            # ---- phase 2: chunked linear attention (both batches interleaved) ----
            with (
                tc.tile_pool(name="psS", bufs=1, space="PSUM") as psS,
                tc.tile_pool(name="psO", bufs=2, space="PSUM") as psO,
                tc.tile_pool(name="psTr", bufs=2, space="PSUM") as psTr,
                tc.tile_pool(name="psF", bufs=2, space="PSUM") as psF,
                tc.tile_pool(name="attn", bufs=4) as attn,
                tc.tile_pool(name="fin", bufs=2) as fin,
            ):
                S = [psS.tile([HD, HD + 1], f32, tag=f"S{b}", name=f"S{b}")
                     for b in range(B)]
                for j in range(CPB):
                    for b in range(B):
                        i = b * CPB + j
                        cs = ts(i, C)
                        # out in NORMAL orientation (t x m+1): denom lands
                        # per-partition so the epilogue is tensor_scalar ops.
                        po = psO.tile([C, HD + 1], f32, tag="po")
                        if j == 0:
                            nc.tensor.matmul(po, sm_all[:, i, :], v_aug[:, i, :],
                                             start=True, stop=True)
                        else:
                            ssb = attn.tile([HD, HD + 1], f32, tag="ssb")
                            nc.scalar.copy(ssb, S[b])
                            nc.tensor.matmul(po, sm_all[:, i, :], v_aug[:, i, :],
                                             start=True, stop=False)
                            nc.tensor.matmul(po, qk_phi[0:64, cs], ssb,
                                             start=False, stop=True)
                        if j < CPB - 1:
                            nc.tensor.matmul(S[b], k_nrm[:, i, :], v_aug[:, i, :],
                                             start=(j == 0), stop=(j == CPB - 2))
                        dn = attn.tile([C, 1], f32, tag="dn")
                        nc.vector.tensor_scalar_max(dn, po[:, HD:HD + 1], 1e-6)
                        dr = attn.tile([C, 1], f32, tag="dr")
                        nc.vector.reciprocal(dr, dn)
                        on = attn.tile([C, HD], f32, tag="on")
                        nc.vector.tensor_scalar_mul(on, po[:, 0:HD], dr)
                        ptr = psTr.tile([HD, C], f32, tag="ptr")
                        nc.tensor.transpose(ptr, on, iden128_sb[:])
                        ot = attn.tile([HD, C], f32, tag="ot")
                        nc.scalar.copy(ot, ptr)
                        nc.sync.dma_start(
                            cc_in[i // 2, :, (i % 2) * C:(i % 2) * C + C], ot)

                # ---- phase 3: A2A + o-projection for my t-slice ----
                nc.gpsimd.collective_compute(
                    "AllToAll",
                    mybir.AluOpType.bypass,
                    replica_groups=[list(range(8))],
                    ins=[cc_in.opt()],
                    outs=[cc_out.opt()],
                )
                # stack head pairs on partitions: K=128 o-proj matmuls
                for h in range(H):
                    nc.sync.dma_start(
                        g_sb[64 * (h % 2):64 * (h % 2) + 64, h // 2, :],
                        cc_out[h, :, :])
                for e in range(4):
                    pf = psF.tile([128, TS], f32)
                    for ki in range(NF):
                        nc.tensor.matmul(pf, wo_sb[:, ki, 128 * e:128 * (e + 1)],
                                         g_sb[:, ki, :],
                                         start=(ki == 0), stop=(ki == NF - 1))
                    osl = fin.tile([128, TS], f32)
                    nc.vector.tensor_scalar_add(osl, pf, bo_sb[:, e:e + 1])
                    nc.sync.dma_start(out[128 * e:128 * (e + 1), :], osl)

    nc.compile()
    _CACHE["nc"] = nc
    return nc


def _in_maps(x, Wq, bq, Wk, bk, Wv, bv, Wo, bo):
    x2 = np.ascontiguousarray(x.reshape(BT, D).T).astype(np.float32)
    woT = np.ascontiguousarray(Wo.T).astype(np.float32)
    bo_c = np.ascontiguousarray(bo.reshape(D, 1)).astype(np.float32)
    maps = []
    for c in range(8):
        sl = slice(HD * c, HD * (c + 1))
        maps.append(dict(
            xT=x2,
            wa=np.ascontiguousarray(np.concatenate([Wq[sl], Wk[sl]], 0).T).astype(np.float32),
            wv=np.ascontiguousarray(Wv[sl].T).astype(np.float32),
            wo=woT,
            bqk=np.ascontiguousarray(np.concatenate([bq[sl], bk[sl]]).reshape(128, 1)).astype(np.float32),
            bv=np.ascontiguousarray(bv[sl].reshape(HD, 1)).astype(np.float32),
            bo=bo_c,
        ))
    return maps


def kernel(x, Wq, bq, Wk, bk, Wv, bv, Wo, bo):
    from concourse import bass_utils

    nc = _build()
    maps = _in_maps(np.asarray(x), np.asarray(Wq), np.asarray(bq),
                    np.asarray(Wk), np.asarray(bk), np.asarray(Wv),
                    np.asarray(bv), np.asarray(Wo), np.asarray(bo))
    res = bass_utils.run_bass_kernel_spmd(nc, maps, core_ids=list(range(8)))
    slices = [res.results[c]["out"] for c in range(8)]   # (512, 256) o.T slices
    o = np.concatenate([s.T for s in slices], axis=0)    # (2048, 512)
    return np.ascontiguousarray(o.reshape(B, T, D)).astype(np.float32)


# revision 27
# speedup vs baseline: 1.8388x; 1.8388x over previous
"""Trainium2 Bass kernel for AssociativeMemoryModule (causal linear attention).

Sharding: head-parallel — core c owns head c for both batches. Each core:
  1. projects full x (pre-transposed on host) to [q.T;k.T] and v.T for its head,
  2. applies phi = elu+1 via exp(min(.,0)) + relu(.),
  3. PE-transposes k_phi / v tiles to normal layout,
  4. runs chunked causal linear attention (C=128) in transposed-output form,
  5. writes per-head out.T blocks to a DRAM bounce in t-blocked layout,
  6. one AllToAll redistributes head-sharded -> t-sharded,
  7. each core computes the o-projection for its 256-row t-slice.
Host gathers the 8 (512, 256) o.T slices and concatenates.
"""
import sys

import numpy as np

sys.path.insert(0, "/opt/trn_rl_repo")

H, HD, D = 8, 64, 512
B, T = 2, 1024
BT = B * T            # 2048
C = 128               # attention chunk
NCH = BT // C         # 16 chunks total
CPB = T // C          # 8 chunks per batch
TS = BT // 8          # 256: per-core output t-slice
NF = D // 128         # 4 feature tiles
NT = 4                # t-tiles of 512 for projections

_CACHE = {}


def _build():
    if "nc" in _CACHE:
        return _CACHE["nc"]
    import concourse.bass as bass
    import concourse.mybir as mybir
    import concourse.tile as tile
    from concourse import bacc
    from concourse.bass import ts

    import ml_dtypes

    f32 = mybir.dt.float32
    bf16 = mybir.dt.bfloat16
    AF = mybir.ActivationFunctionType

    nc = bacc.Bacc("TRN2", target_bir_lowering=False, debug=False, num_devices=8)

    xT = nc.declare_dram_parameter("xT", [D, BT], bf16, isOutput=False)
    wa = nc.declare_dram_parameter("wa", [D, 128], bf16, isOutput=False)
    wv = nc.declare_dram_parameter("wv", [D, HD], bf16, isOutput=False)
    wo = nc.declare_dram_parameter("wo", [D, D], bf16, isOutput=False)
    bqk = nc.declare_dram_parameter("bqk", [128, 1], f32, isOutput=False)
    bv = nc.declare_dram_parameter("bv", [HD, 1], f32, isOutput=False)
    bo = nc.declare_dram_parameter("bo", [D, 1], f32, isOutput=False)
    out = nc.declare_dram_parameter("out", [D, TS], f32, isOutput=True)

    mask_np = np.triu(np.ones((C, C), np.float32))  # mask[s,t] = s<=t
    iden_np = np.eye(HD, dtype=ml_dtypes.bfloat16)
    iden128_np = np.eye(C, dtype=ml_dtypes.bfloat16)
    mask_d = nc.inline_tensor(mask_np, "causal_mask")
    iden_d = nc.inline_tensor(iden_np, "iden64")
    iden128_d = nc.inline_tensor(iden128_np, "iden128")

    with tile.TileContext(nc) as tc:
        with (
            tc.tile_pool(name="consts", bufs=1) as consts,
            tc.tile_pool(name="dram", bufs=1, space="DRAM") as dram,
        ):
            # ---- resident SBUF tensors (matmul operands in bf16) ----
            xt_sb = consts.tile([128, NF, BT], bf16)
            wa_sb = consts.tile([128, NF, 128], bf16)
            wv_sb = consts.tile([128, NF, HD], bf16)
            wo_sb = consts.tile([128, NF, D], bf16)
            bqk_sb = consts.tile([128, 1], f32)
            bv_sb = consts.tile([HD, 1], f32)
            bo_sb = consts.tile([128, NF], f32)
            mask_sb = consts.tile([C, C], f32)
            iden_sb = consts.tile([HD, HD], bf16)
            iden128_sb = consts.tile([C, C], bf16)
            qk_phi = consts.tile([128, BT], bf16)      # rows 0-63 qT, 64-127 kT
            k_sep = consts.tile([HD, BT], bf16)        # kT re-based to partition 0
            vT_sb = consts.tile([HD, BT], bf16)
            k_nrm = consts.tile([128, NCH, HD], bf16)
            v_aug = consts.tile([128, NCH, HD + 1], bf16)
            g_sb = consts.tile([128, NF, TS], bf16)
            sm_all = consts.tile([C, NCH, C], bf16)

            cc_in = dram.tile([8, HD, TS], bf16)       # t-blocked out.T
            cc_out = dram.tile([8, HD, TS], bf16)      # after A2A: head-blocked

            for f in range(NF):
                nc.sync.dma_start(wa_sb[:, f, :], wa[128 * f:128 * (f + 1), :])
                nc.sync.dma_start(wv_sb[:, f, :], wv[128 * f:128 * (f + 1), :])
            # xT split by (t-col, f) chunk so t-tile 0 becomes computable fast
            for tcol in range(NT):
                for f in range(NF):
                    nc.sync.dma_start(
                        xt_sb[:, f, ts(tcol, 512)],
                        xT[128 * f:128 * (f + 1), ts(tcol, 512)])
            for f in range(NF):
                nc.sync.dma_start(wo_sb[:, f, :], wo[128 * f:128 * (f + 1), :])
            nc.sync.dma_start(bqk_sb[:], bqk[:, :])
            nc.sync.dma_start(bv_sb[:], bv[:, :])
            nc.sync.dma_start(bo_sb[:], bo.ap().rearrange("(e p) o -> p (e o)", p=128))
            nc.sync.dma_start(mask_sb[:], mask_d[:, :])
            nc.sync.dma_start(iden_sb[:], iden_d[:, :])
            nc.sync.dma_start(iden128_sb[:], iden128_d[:, :])
            nc.vector.memset(v_aug[:, :, HD:HD + 1], 1.0)

            # ---- phase 1: projections + phi + transposes + scores ----
            with (
                tc.tile_pool(name="psA", bufs=2, space="PSUM") as psA,
                tc.tile_pool(name="psB", bufs=2, space="PSUM") as psB,
                tc.tile_pool(name="psT", bufs=2, space="PSUM") as psT,
                tc.tile_pool(name="psSc", bufs=2, space="PSUM") as psSc,
                tc.tile_pool(name="ptmp", bufs=2) as ptmp,
            ):
                for tt in range(NT):
                    sl = ts(tt, 512)
                    pa = psA.tile([128, 512], f32)
                    pb = psB.tile([HD, 512], f32)
                    for f in range(NF):
                        nc.tensor.matmul(pa, wa_sb[:, f, :], xt_sb[:, f, sl],
                                         start=(f == 0), stop=(f == NF - 1))
                    for f in range(NF):
                        nc.tensor.matmul(pb, wv_sb[:, f, :], xt_sb[:, f, sl],
                                         start=(f == 0), stop=(f == NF - 1))
                    qkr = ptmp.tile([128, 512], f32, tag="qkr")
                    nc.scalar.activation(qkr, pa, AF.Identity, bias=bqk_sb[:])
                    nc.scalar.activation(vT_sb[:, sl], pb, AF.Identity, bias=bv_sb[:])
                    mm = ptmp.tile([128, 512], f32, tag="mm")
                    rr = ptmp.tile([128, 512], f32, tag="rr")
                    ee = ptmp.tile([128, 512], f32, tag="ee")
                    nc.vector.tensor_scalar_min(mm, qkr, 0.0)
                    nc.scalar.activation(ee, mm, AF.Exp)
                    nc.vector.tensor_scalar_max(rr, qkr, 0.0)
                    nc.vector.tensor_add(qk_phi[:, sl], ee, rr)
                    # re-base kT rows 64-127 -> partition 0 (SBUF->SBUF DMA)
                    nc.sync.dma_start(k_sep[:, sl], qk_phi[64:128, sl])

                    # transposes + masked scores for the 4 chunks in this t-tile
                    for jj in range(4):
                        i = tt * 4 + jj
                        cs = ts(i, C)
                        pt = psT.tile([C, HD], bf16, tag="tr")
                        nc.tensor.transpose(pt, k_sep[:, cs], iden_sb[:])
                        nc.scalar.copy(k_nrm[:, i, :], pt)
                        pv = psT.tile([C, HD], bf16, tag="tr")
                        nc.tensor.transpose(pv, vT_sb[:, cs], iden_sb[:])
                        nc.vector.tensor_copy(v_aug[:, i, 0:HD], pv)
                        ps = psSc.tile([C, C], f32, tag="ps")
                        nc.tensor.matmul(ps, k_sep[:, cs], qk_phi[0:64, cs],
                                         start=True, stop=True)
                        nc.vector.tensor_mul(sm_all[:, i, :], ps, mask_sb[:])

            # ---- phase 2: chunked linear attention (both batches interleaved) ----
            with (
                tc.tile_pool(name="psS", bufs=1, space="PSUM") as psS,
                tc.tile_pool(name="psO", bufs=2, space="PSUM") as psO,
                tc.tile_pool(name="psTr", bufs=2, space="PSUM") as psTr,
                tc.tile_pool(name="psF", bufs=2, space="PSUM") as psF,
                tc.tile_pool(name="attn", bufs=4) as attn,
                tc.tile_pool(name="fin", bufs=2) as fin,
            ):
                S = [psS.tile([HD, HD + 1], f32, tag=f"S{b}", name=f"S{b}")
                     for b in range(B)]
                for j in range(CPB):
                    for b in range(B):
                        i = b * CPB + j
                        cs = ts(i, C)
                        # out in NORMAL orientation (t x m+1): denom lands
                        # per-partition so the epilogue is tensor_scalar ops.
                        po = psO.tile([C, HD + 1], f32, tag="po")
                        if j == 0:
                            nc.tensor.matmul(po, sm_all[:, i, :], v_aug[:, i, :],
                                             start=True, stop=True)
                        else:
                            ssb = attn.tile([HD, HD + 1], bf16, tag="ssb")
                            nc.scalar.copy(ssb, S[b])
                            nc.tensor.matmul(po, sm_all[:, i, :], v_aug[:, i, :],
                                             start=True, stop=False)
                            nc.tensor.matmul(po, qk_phi[0:64, cs], ssb,
                                             start=False, stop=True)
                        if j < CPB - 1:
                            nc.tensor.matmul(S[b], k_nrm[:, i, :], v_aug[:, i, :],
                                             start=(j == 0), stop=(j == CPB - 2))
                        dn = attn.tile([C, 1], f32, tag="dn")
                        nc.vector.tensor_scalar_max(dn, po[:, HD:HD + 1], 1e-6)
                        dr = attn.tile([C, 1], f32, tag="dr")
                        nc.vector.reciprocal(dr, dn)
                        on = attn.tile([C, HD], bf16, tag="on")
                        nc.vector.tensor_scalar_mul(on, po[:, 0:HD], dr)
                        ptr = psTr.tile([HD, C], bf16, tag="ptr")
                        nc.tensor.transpose(ptr, on, iden128_sb[:])
                        ot = attn.tile([HD, C], bf16, tag="ot")
                        nc.scalar.copy(ot, ptr)
                        nc.sync.dma_start(
                            cc_in[i // 2, :, (i % 2) * C:(i % 2) * C + C], ot)

                # ---- phase 3: A2A + o-projection for my t-slice ----
                nc.gpsimd.collective_compute(
                    "AllToAll",
                    mybir.AluOpType.bypass,
                    replica_groups=[list(range(8))],
                    ins=[cc_in.opt()],
                    outs=[cc_out.opt()],
                )
                # stack head pairs on partitions: K=128 o-proj matmuls
                for h in range(H):
                    nc.sync.dma_start(
                        g_sb[64 * (h % 2):64 * (h % 2) + 64, h // 2, :],
                        cc_out[h, :, :])
                for e in range(4):
                    pf = psF.tile([128, TS], f32)
                    for ki in range(NF):
                        nc.tensor.matmul(pf, wo_sb[:, ki, 128 * e:128 * (e + 1)],
                                         g_sb[:, ki, :],
                                         start=(ki == 0), stop=(ki == NF - 1))
                    osl = fin.tile([128, TS], f32)
                    nc.vector.tensor_scalar_add(osl, pf, bo_sb[:, e:e + 1])
                    nc.sync.dma_start(out[128 * e:128 * (e + 1), :], osl)

    nc.compile()
    _CACHE["nc"] = nc
    return nc


def _in_maps(x, Wq, bq, Wk, bk, Wv, bv, Wo, bo):
    import ml_dtypes
    bf = ml_dtypes.bfloat16
    x2 = np.ascontiguousarray(x.reshape(BT, D).T).astype(bf)
    woT = np.ascontiguousarray(Wo.T).astype(bf)
    bo_c = np.ascontiguousarray(bo.reshape(D, 1)).astype(np.float32)
    maps = []
    for c in range(8):
        sl = slice(HD * c, HD * (c + 1))
        maps.append(dict(
            xT=x2,
            wa=np.ascontiguousarray(np.concatenate([Wq[sl], Wk[sl]], 0).T).astype(bf),
            wv=np.ascontiguousarray(Wv[sl].T).astype(bf),
            wo=woT,
            bqk=np.ascontiguousarray(np.concatenate([bq[sl], bk[sl]]).reshape(128, 1)).astype(np.float32),
            bv=np.ascontiguousarray(bv[sl].reshape(HD, 1)).astype(np.float32),
            bo=bo_c,
        ))
    return maps


def kernel(x, Wq, bq, Wk, bk, Wv, bv, Wo, bo):
    from concourse import bass_utils

    nc = _build()
    maps = _in_maps(np.asarray(x), np.asarray(Wq), np.asarray(bq),
                    np.asarray(Wk), np.asarray(bk), np.asarray(Wv),
                    np.asarray(bv), np.asarray(Wo), np.asarray(bo))
    res = bass_utils.run_bass_kernel_spmd(nc, maps, core_ids=list(range(8)))
    slices = [res.results[c]["out"] for c in range(8)]   # (512, 256) o.T slices
    o = np.concatenate([s.T for s in slices], axis=0)    # (2048, 512)
    return np.ascontiguousarray(o.reshape(B, T, D)).astype(np.float32)
